# revision 1
# baseline (speedup 1.0000x reference)
"""LoRA multi-head attention kernel for 8 Trainium2 NeuronCores.

Problem: q = x_q@(Wq.T + Aq@Bq*2) + bq ; k = x_k@Wk.T + bk ;
         v = x_v@(Wv.T + Av@Bv*2) + bv ; MHA over 16 heads, D=64,
         out = attn_out @ Wo.T + bo.   Shapes: x [2048, 4, 1024].

Sharding: core c handles batch b = c//2 and head-group hg = c%2
(8 heads = 512 channels). LoRA weights are merged on the host
(mathematically exact), the 1/sqrt(D) score scale is folded into Wk/bk,
and x is transposed on the host so every matmul contracts over the
partition dimension. Each core computes a partial output
(its 512 channels through Wo); the host sums the two partials per batch.

Device layout per core:
  qT/kT  [ch, tok] ; v [tok, ch] augmented with a ones column so the
  attn@v matmul also produces the softmax denominator (scores are
  exponentiated WITHOUT max subtraction -- safe here, |scores| < ~6 --
  and normalization happens after attn@v on the [D, S] output, 32x
  cheaper than normalizing the attention matrix).
All matmuls run as float32r (full PE rate at free dim 512).
"""

import sys

import numpy as np

sys.path.insert(0, "/opt/trn_rl_repo")

from contextlib import ExitStack  # noqa: E402

import concourse.bass as bass  # noqa: E402
import concourse.tile as tile  # noqa: E402
from concourse import bacc, mybir  # noqa: E402
from concourse.bass_utils import run_bass_kernel_spmd  # noqa: E402

F32 = mybir.dt.float32
F32R = mybir.dt.float32r
AF = mybir.ActivationFunctionType
ALU = mybir.AluOpType

E = 1024
D = 64
NHC = 8            # heads per core
CH = NHC * D       # 512 output channels per core
KT = E // 128      # k-tiles over the E contraction
NCORES = 8
B = 4


def build_program(S=2048, num_devices=8):
    TB = 256 if S >= 512 else S     # token block for projections
    NTB = S // TB
    NSB = S // 512 if S >= 512 else 1
    SBK = S // NSB                  # s-block width
    NTT = S // 128                  # t tiles
    MT = S // 128                   # tok tiles (v projection / output)
    NM = CH // 128                  # ch tiles per core (4)

    nc = bacc.Bacc(
        "TRN2", target_bir_lowering=False, debug=False, num_devices=num_devices
    )

    def dram(name, shape, out=False, dt=F32):
        kind = "ExternalOutput" if out else "ExternalInput"
        return nc.dram_tensor(name, shape, dt, kind=kind).ap()

    xq = dram("xq", [128, KT, S], dt=F32R)
    xk = dram("xk", [128, KT, S], dt=F32R)
    xv = dram("xv", [128, KT, S], dt=F32R)
    wq = dram("wq", [128, KT, CH], dt=F32R)
    wk = dram("wk", [128, KT, CH], dt=F32R)
    wv = dram("wv", [128, KT, CH], dt=F32R)
    wo = dram("wo", [128, NM, E // 512, 512], dt=F32R)
    bq = dram("bq", [128, NM])
    bk = dram("bk", [128, NM])
    bv = dram("bv", [128, CH])
    bo = dram("bo", [128, E])
    onesd = dram("onesd", [64], dt=F32R)
    out = dram("out", [S, E], out=True)

    with tile.TileContext(nc) as tc, ExitStack() as top:
        persist = top.enter_context(tc.tile_pool(name="persist", bufs=1))
        qT = persist.tile([128, NM, S], F32R)          # [ch%128, ch//128, tok]
        kT = persist.tile([128, NM, S], F32R)
        vaug = persist.tile([128, NTT, NHC, D + 1], F32R)  # [tok%128, ttile, h, d+1]
        aoT = persist.tile([128, NM, S], F32R)         # attention out, [ch, tok]
        bq_sb = persist.tile([128, NM], F32)
        bk_sb = persist.tile([128, NM], F32)
        bv_sb = persist.tile([128, CH], F32)
        ones_sb = persist.tile([1, D], F32R)
        nc.sync.dma_start(out=bq_sb, in_=bq)
        nc.sync.dma_start(out=bk_sb, in_=bk)
        nc.sync.dma_start(out=bv_sb, in_=bv)
        nc.gpsimd.dma_start(out=ones_sb, in_=onesd[None, :])
        nc.vector.memset(vaug[:, :, :, D:D + 1].bitcast(F32), 1.0)

        # ---------------- Phase A: q/k/v projections ----------------
        with tc.tile_pool(name="wts", bufs=1) as wpool, \
             tc.tile_pool(name="xs", bufs=3) as xpool, \
             tc.tile_pool(name="pps", bufs=3, space="PSUM") as ppool:
            wq_sb = wpool.tile([128, KT, CH], F32R, tag="wq")
            wk_sb = wpool.tile([128, KT, CH], F32R, tag="wk")
            wv_sb = wpool.tile([128, KT, CH], F32R, tag="wv")
            nc.sync.dma_start(out=wq_sb, in_=wq)
            nc.sync.dma_start(out=wk_sb, in_=wk)
            nc.sync.dma_start(out=wv_sb, in_=wv)

            # k then q: qT/kT[ch, tok] = W.T @ x.T  (+ bias per partition)
            for xap, w_sb, b_sb, dst in (
                (xk, wk_sb, bk_sb, kT),
                (xq, wq_sb, bq_sb, qT),
            ):
                for nb in range(NTB):
                    xt = xpool.tile([128, KT, TB], F32R, tag="x")
                    nc.sync.dma_start(out=xt, in_=xap[:, :, nb * TB:(nb + 1) * TB])
                    for m in range(NM):
                        ps = ppool.tile([128, TB], F32, tag="pp")
                        for k in range(KT):
                            nc.tensor.matmul(
                                ps,
                                (w_sb[:, k, m * 128:(m + 1) * 128]),
                                (xt[:, k, :]),
                                start=(k == 0),
                                stop=(k == KT - 1),
                            )
                        nc.vector.tensor_scalar(
                            out=dst[:, m, nb * TB:(nb + 1) * TB],
                            in0=ps,
                            scalar1=b_sb[:, m:m + 1],
                            scalar2=None,
                            op0=ALU.add,
                        )
            # v: v[tok, ch] = x @ Wv_eff  (+ bias along free dim)
            for nb in range(NTB):
                xt = xpool.tile([128, KT, TB], F32R, tag="x")
                nc.sync.dma_start(out=xt, in_=xv[:, :, nb * TB:(nb + 1) * TB])
                for mi in range(TB // 128):
                    mt = nb * (TB // 128) + mi
                    ps = ppool.tile([128, CH], F32, tag="pp")
                    for k in range(KT):
                        nc.tensor.matmul(
                            ps,
                            (xt[:, k, mi * 128:(mi + 1) * 128]),
                            (wv_sb[:, k, :]),
                            start=(k == 0),
                            stop=(k == KT - 1),
                        )
                    nc.vector.tensor_add(
                        out=vaug[:, mt, :, 0:D],
                        in0=ps.rearrange("p (h d) -> p h d", d=D),
                        in1=bv_sb.rearrange("p (h d) -> p h d", d=D),
                    )

        # ---------------- Phase B: attention ----------------
        # scores_T[t, s] = k_scaled @ q.T per head; exp; oaug = [v | 1].T @ exp
        # (row D of oaug = softmax denominator); normalize into aoT.
        with tc.tile_pool(name="scps", bufs=1, space="PSUM") as scpool, \
             tc.tile_pool(name="oaps", bufs=1, space="PSUM") as opool, \
             tc.tile_pool(name="bcps", bufs=1, space="PSUM") as bcpool, \
             tc.tile_pool(name="exs", bufs=4) as expool, \
             tc.tile_pool(name="nrm", bufs=3) as npool:
            for hp in range(NM):
                for sb_i in range(NSB):
                    ssl = slice(sb_i * SBK, (sb_i + 1) * SBK)
                    oaugs = [
                        opool.tile(
                            [D + 1, SBK], F32, tag=f"oaug{h_in}", name=f"oaug{h_in}"
                        )
                        for h_in in range(2)
                    ]
                    for tt2 in range(NTT // 2):
                        for h_in in range(2):
                            h = 2 * hp + h_in
                            p0 = h_in * 64
                            sc = scpool.tile([128, 2, SBK], F32, tag=f"sc{h_in}")
                            for j in range(2):
                                tt = tt2 * 2 + j
                                nc.tensor.matmul(
                                    sc[:, j, :],
                                    (kT[p0:p0 + 64, hp, tt * 128:(tt + 1) * 128]),
                                    (qT[p0:p0 + 64, hp, ssl]),
                                    start=True,
                                    stop=True,
                                )
                            ex = expool.tile([128, 2, SBK], F32R, tag=f"ex{h_in}")
                            nc.scalar.activation(out=ex, in_=sc, func=AF.Exp)
                            for j in range(2):
                                tt = tt2 * 2 + j
                                nc.tensor.matmul(
                                    oaugs[h_in],
                                    (vaug[:, tt, h, :]),
                                    (ex[:, j, :]),
                                    start=(tt == 0),
                                    stop=(tt == NTT - 1),
                                )
                    for h_in in range(2):
                        p0 = h_in * 64
                        recip32 = npool.tile([1, SBK], F32, tag="recip32")
                        nc.vector.reciprocal(out=recip32, in_=oaugs[h_in][D:D + 1, :])
                        recip = npool.tile([1, SBK], F32R, tag="recip")
                        nc.vector.tensor_copy(out=recip, in_=recip32)
                        bc = bcpool.tile([D, SBK], F32, tag="bc")
                        nc.tensor.matmul(
                            bc, (ones_sb), (recip), start=True, stop=True
                        )
                        rb = npool.tile([D, SBK], F32, tag="rb")
                        nc.vector.tensor_copy(out=rb, in_=bc)
                        nc.vector.tensor_mul(
                            out=aoT[p0:p0 + 64, hp, ssl],
                            in0=oaugs[h_in][0:D, :],
                            in1=rb,
                        )

        # ---------------- Phase C: output projection (partial Wo) ----------------
        with tc.tile_pool(name="wos", bufs=1) as wopool, \
             tc.tile_pool(name="wops", bufs=2, space="PSUM") as wpp, \
             tc.tile_pool(name="outs", bufs=3) as outpool:
            wo_sb = wopool.tile([128, NM, E // 512, 512], F32R)
            bo_sb = wopool.tile([128, E], F32)
            nc.sync.dma_start(out=wo_sb, in_=wo)
            nc.sync.dma_start(out=bo_sb, in_=bo)
            for mt in range(MT):
                for nb2 in range(E // 512):
                    ps = wpp.tile([128, 512], F32, tag="wops")
                    for kc in range(NM):
                        nc.tensor.matmul(
                            ps,
                            (aoT[:, kc, mt * 128:(mt + 1) * 128]),
                            (wo_sb[:, kc, nb2, :]),
                            start=(kc == 0),
                            stop=(kc == NM - 1),
                        )
                    ot = outpool.tile([128, 512], F32, tag="ot")
                    nc.vector.tensor_add(
                        out=ot, in0=ps, in1=bo_sb[:, nb2 * 512:(nb2 + 1) * 512]
                    )
                    nc.sync.dma_start(
                        out=out[mt * 128:(mt + 1) * 128, nb2 * 512:(nb2 + 1) * 512],
                        in_=ot,
                    )

    nc.compile()
    return nc


_PROG = {}


def _get_prog(S=2048, num_devices=8):
    key = (S, num_devices)
    if key not in _PROG:
        _PROG[key] = build_program(S, num_devices)
    return _PROG[key]


def _tile_x(x2d):
    # [S, E] slice -> [128, KT, S] with element (p, k, t) = x2d[t, k*128+p]
    S = x2d.shape[0]
    xt = np.ascontiguousarray(x2d.T.astype(np.float32))
    return np.ascontiguousarray(xt.reshape(KT, 128, S).transpose(1, 0, 2))


def _tile_w(weff, ch0):
    w = weff[:, ch0:ch0 + CH]
    return np.ascontiguousarray(
        w.reshape(KT, 128, CH).transpose(1, 0, 2).astype(np.float32)
    )


def prep_in_maps(x_q, x_k, x_v, Wq, bq, Aq, Bq, Wk, bk, Wv, bv, Av, Bv, Wo, bo):
    x_q = np.asarray(x_q, np.float32)
    x_k = np.asarray(x_k, np.float32)
    x_v = np.asarray(x_v, np.float32)
    scaling = 2.0  # lora_alpha / r = 32 / 16
    wq_eff = (np.asarray(Wq).T + (np.asarray(Aq) @ np.asarray(Bq)) * scaling).astype(
        np.float32
    )
    wv_eff = (np.asarray(Wv).T + (np.asarray(Av) @ np.asarray(Bv)) * scaling).astype(
        np.float32
    )
    wk_s = (np.asarray(Wk).T / 8.0).astype(np.float32)  # sqrt(D) folded in
    bk_s = (np.asarray(bk) / 8.0).astype(np.float32)
    bq = np.asarray(bq, np.float32)
    bv = np.asarray(bv, np.float32)
    bo = np.asarray(bo, np.float32)
    woT = np.ascontiguousarray(np.asarray(Wo).T.astype(np.float32))

    nbatch = x_q.shape[1]
    in_maps = []
    for c in range(2 * nbatch):
        b = c // 2
        hg = c % 2
        ch0 = hg * CH
        wo_c = np.ascontiguousarray(
            woT[ch0:ch0 + CH, :].reshape(CH // 128, 128, E // 512, 512)
            .transpose(1, 0, 2, 3)
        )
        in_maps.append({
            "xq": _tile_x(x_q[:, b, :]),
            "xk": _tile_x(x_k[:, b, :]),
            "xv": _tile_x(x_v[:, b, :]),
            "wq": _tile_w(wq_eff, ch0),
            "wk": _tile_w(wk_s, ch0),
            "wv": _tile_w(wv_eff, ch0),
            "wo": wo_c,
            "bq": np.ascontiguousarray(bq[ch0:ch0 + CH].reshape(CH // 128, 128).T),
            "bk": np.ascontiguousarray(bk_s[ch0:ch0 + CH].reshape(CH // 128, 128).T),
            "bv": np.ascontiguousarray(np.broadcast_to(bv[ch0:ch0 + CH], (128, CH))),
            "onesd": np.ones(64, np.float32),
            "bo": (
                np.ascontiguousarray(np.broadcast_to(bo, (128, E)))
                if hg == 0
                else np.zeros((128, E), np.float32)
            ),
        })
    return in_maps


def gather_out(results, nbatch):
    return np.stack(
        [results[2 * b]["out"] + results[2 * b + 1]["out"] for b in range(nbatch)],
        axis=1,
    )


def kernel(**inputs):
    nc = _get_prog(2048, 8)
    in_maps = prep_in_maps(**inputs)
    res = run_bass_kernel_spmd(nc, in_maps, core_ids=list(range(NCORES)))
    return gather_out(res.results, B)



# revision 26
# speedup vs baseline: 1.4640x; 1.4640x over previous
"""LoRA multi-head attention kernel for 8 Trainium2 NeuronCores (v3).

Math: q = x_q@(Wq.T + Aq@Bq*2) + bq ; k = x_k@Wk.T (bk dropped: its score
term is constant over keys -> softmax-invariant) ; v = x_v@(Wv.T + Av@Bv*2)
+ bv ; 16-head attention, D=64; out = attn_out @ Wo.T + bo.

Sharding: core c = (batch b=c//2, head-group hg=c%2) -> 8 heads, 512
channels. LoRA merged on host (exact). Each core computes a partial output
(its 512 channels through Wo); host sums the two partials per batch.

Engine plan (TimelineSim cost model): ACT exp over the 8x2048x2048 score
tiles is the bottleneck (~0.83 ns/elem). Everything else is shaped to keep
ACT 100% busy:
  - scores in fp8e4 + DoubleRow ([32, 2-slab] layout), q/k projections in
    fp8e4 + DoubleRow (2x128 contraction per matmul).
  - attn@v in bf16 with OUTPUT [s-chunk=128 part, d+1]: full 128 output
    partitions per streamed column; softmax denominator rides in column 64.
  - per-head normalize via per-partition tensor_scalar; PE-transpose
    normalized [s, ch] -> aoT [ch, s]; bf16 output projection.
  - PSUM banks (groups zero a full 2 KiB bank on start -> one open group
    per bank): sc ring 3+2 banks (alternating chunk sizes), 1 acc bank
    (one open group per unit), 2 flex banks shared by paced projection /
    transpose / outproj groups in strict A/B alternation.
  - phase B starts right after k(g0)+q(g0,nb0); the remaining projections
    (q rest, k/q g1, all of v in head-pair-major order) are paced into the
    chunk stream ahead of static deadlines so ACT never waits.
"""

import os
import sys

import numpy as np

sys.path.insert(0, "/opt/trn_rl_repo")

NODR_PROJ = bool(int(os.environ.get("NODR_PROJ", "0")))
NODR_SCORES = bool(int(os.environ.get("NODR_SCORES", "0")))
F32T = bool(int(os.environ.get("F32T", "0")))
DBG = bool(int(os.environ.get("DBG", "0")))
QK16 = bool(int(os.environ.get("QK16", "0")))

from collections import deque  # noqa: E402
from contextlib import ExitStack  # noqa: E402

import ml_dtypes  # noqa: E402

import concourse.bass as bass  # noqa: E402
import concourse.tile as tile  # noqa: E402
from concourse import bacc, mybir  # noqa: E402
from concourse.bass_utils import run_bass_kernel_spmd  # noqa: E402

F32 = mybir.dt.float32
BF16 = mybir.dt.bfloat16
FP8 = mybir.dt.float8e4
AF = mybir.ActivationFunctionType
ALU = mybir.AluOpType
DR = mybir.MatmulPerfMode.DoubleRow
QKDT = BF16 if QK16 else FP8

E = 1024
D = 64
NHC = 8            # heads per core
CH = NHC * D       # 512 output channels per core
KT = E // 128      # 8 k-tiles over the E contraction
NCORES = 8
B = 4
S = 2048
NTT = S // 128     # 16 t tiles
NSB = S // 512     # 4 s blocks
NM = CH // 128     # 4 ch tiles per projection
TB = 512           # projection token block
NTB = S // TB
NU = NSB * NHC     # 32 (sblock, head) units
NGT = NU * NTT     # 512 score tiles of [128, 512]
EXBUFS = 10


def _chunk_list():
    """Alternating 3/2-tile chunks over the NGT score tiles."""
    chunks = []
    gt = 0
    size = 3
    while gt < NGT:
        n = min(size, NGT - gt)
        chunks.append(list(range(gt, gt + n)))
        gt += n
        size = 5 - size
    return chunks


def build_program(num_devices=8):
    nc = bacc.Bacc(
        "TRN2", target_bir_lowering=False, debug=False, num_devices=num_devices
    )

    def dram(name, shape, dt=F32, out=False):
        kind = "ExternalOutput" if out else "ExternalInput"
        return nc.dram_tensor(name, shape, dt, kind=kind).ap()

    xq8 = dram("xq8", [128, KT, S], QKDT)
    xk8 = dram("xk8", [128, KT, S], QKDT)
    xv16 = dram("xv16", [128, KT, S], BF16)
    wq8 = dram("wq8", [128, 2, KT, CH // 2], QKDT)
    wk8 = dram("wk8", [128, 2, KT, CH // 2], QKDT)
    wv16 = dram("wv16", [128, KT, CH], BF16)
    wo16 = dram("wo16", [128, NM, E], BF16)
    bq = dram("bq", [128, NM])
    bv16 = dram("bv16", [128, CH], BF16)
    bo16 = dram("bo16", [128, E], BF16)
    ident = dram("ident", [128, 128], BF16)
    out = dram("out", [S, E], out=True)
    if DBG:
        d_qT = dram("d_qT", [128, 2, 2, S], FP8, out=True)
        d_kT = dram("d_kT", [128, 2, 2, S], FP8, out=True)
        d_vaug = dram("d_vaug", [128, NTT, NHC, D + 1], BF16, out=True)
        d_aoT = dram("d_aoT", [128, NM, S], BF16, out=True)

    chunks = _chunk_list()
    NCH = len(chunks)
    # chunk index that finishes each unit (contains gt = u*16+15)
    unit_end_chunk = {}
    for ci, gts in enumerate(chunks):
        for gt in gts:
            if gt % NTT == NTT - 1:
                unit_end_chunk[gt // NTT] = ci

    # ---- static pacing schedule for leftover projection work ----
    # paced items (emitted through the 2 flex psum banks):
    #   v groups:  (cb, mt) for cb in 0..3, mt in 0..15   deadline per cb
    #   kq g1:     (proj, m, nb) for m in 2,3             deadline c(u4)
    #   q g0 rest: (q, m in 0..1, nb in 1..3)             deadline per sb
    def c_of_u(u):
        return unit_end_chunk.get(u, NCH - 1)

    paced = []  # (deadline_chunk, kind, args)
    for nb in range(1, NTB):
        dl = max(0, c_of_u(nb * NHC) - NHC * 3)  # before sblock nb begins
        for m in range(2):
            paced.append((dl, "qproj", (m, nb)))
    # k g0 for tokens nb>=1: needed by sblock-0 scores at t-tile 4*nb
    for nb in range(1, NTB):
        first_gt = 4 * nb
        dl = max(0, first_gt // 5 * 2 + (1 if first_gt % 5 >= 3 else 0) - 1)
        for m in range(2):
            paced.append((dl, "kproj", (m, nb)))
    dl_g1 = max(0, c_of_u(3) - 4)  # before first g1 head (u4) scores
    for m in range(2, NM):
        for nb in range(NTB):
            paced.append((dl_g1, "kproj", (m, nb)))
            paced.append((dl_g1, "qproj", (m, nb)))
    # v deadlines: cb's first consumer unit (2*cb) starts pinning ex tiles
    # at the end of unit 2*cb-1; all of cb must be emitted within the ex
    # ring window from there.
    for cb in range(4):
        start_c = 0 if cb == 0 else c_of_u(2 * cb - 1)
        dl = max(2, start_c + EXBUFS - 4)
        for mt in range(NTT):
            paced.append((dl, "vproj", (cb, mt)))
    paced.sort(key=lambda t: t[0])
    # assign each paced item an emission chunk: spread from 0 to deadline
    # greedily in deadline order, at most MAXP per chunk
    MAXP = 2
    slots = [0] * NCH
    emit_at = [[] for _ in range(NCH)]
    for dl, kind, args in paced:
        c = 0
        while slots[c] >= MAXP and c < dl:
            c += 1
        slots[c] += 1
        emit_at[c].append((kind, args))

    with tile.TileContext(nc) as tc, ExitStack() as top:
        persist = top.enter_context(tc.tile_pool(name="persist", bufs=1))
        qT8 = persist.tile([128, 2, 2, S], QKDT)     # [hb*32+r, g, slab, s]
        kT8 = persist.tile([128, 2, 2, S], QKDT)
        vaug = persist.tile([128, NTT, NHC, D + 1], BF16)
        aoT = persist.tile([128, NM, S], BF16)       # [ch%128, ch//128, tok]
        wq_sb = persist.tile([128, 2, KT, CH // 2], QKDT)
        wk_sb = persist.tile([128, 2, KT, CH // 2], QKDT)
        wv_sb = persist.tile([128, KT, CH], BF16)
        wo_sb = persist.tile([128, NM, E], BF16)
        bq_sb = persist.tile([128, NM], F32)
        bv_sb = persist.tile([128, CH], BF16)
        bo_sb = persist.tile([128, E], BF16)
        id_sb = persist.tile([128, 128], BF16)
        id32_sb = persist.tile([128, 128], F32)
        xq_sb = persist.tile([128, KT, S], FP8)
        xk_sb = persist.tile([128, KT, S], FP8)
        xv_sb = persist.tile([128, KT, S], BF16)

        # DMA order = earliest-need order (halved w tensors keep each DMA
        # contiguous at full bus width).
        nc.sync.dma_start(out=wk_sb[:, 0], in_=wk8[:, 0])
        nc.sync.dma_start(out=xk_sb[:, :, 0:TB], in_=xk8[:, :, 0:TB])
        nc.sync.dma_start(out=wq_sb[:, 0], in_=wq8[:, 0])
        nc.sync.dma_start(out=xq_sb[:, :, 0:TB], in_=xq8[:, :, 0:TB])
        nc.sync.dma_start(out=bq_sb, in_=bq)
        for i in range(1, NTB):
            sl = slice(i * TB, (i + 1) * TB)
            nc.sync.dma_start(out=xk_sb[:, :, sl], in_=xk8[:, :, sl])
        nc.sync.dma_start(out=wv_sb, in_=wv16)
        nc.sync.dma_start(out=bv_sb, in_=bv16)
        for i in range(NTB):
            sl = slice(i * TB, (i + 1) * TB)
            nc.sync.dma_start(out=xv_sb[:, :, sl], in_=xv16[:, :, sl])
        nc.sync.dma_start(out=id_sb, in_=ident)
        nc.sync.dma_start(out=wk_sb[:, 1], in_=wk8[:, 1])
        nc.sync.dma_start(out=wq_sb[:, 1], in_=wq8[:, 1])
        for i in range(1, NTB):
            sl = slice(i * TB, (i + 1) * TB)
            nc.sync.dma_start(out=xq_sb[:, :, sl], in_=xq8[:, :, sl])
        nc.sync.dma_start(out=wo_sb, in_=wo16)
        nc.sync.dma_start(out=bo_sb, in_=bo16)
        nc.vector.memset(vaug[:, :, :, D:D + 1], 1.0)
        if F32T:
            nc.vector.tensor_copy(out=id32_sb, in_=id_sb)

        def qk_group(ps, w_sb, x_sb, dstT, m, nb, has_bias):
            g, slab = m // 2, m % 2
            half, mh = m // 2, m % 2
            tsl = slice(nb * TB, (nb + 1) * TB)
            if NODR_PROJ or QK16:
                for kt in range(KT):
                    nc.tensor.matmul(
                        ps,
                        (w_sb[:, half, kt, mh * 128:(mh + 1) * 128]),
                        (x_sb[:, kt, tsl]),
                        start=(kt == 0),
                        stop=(kt == KT - 1),
                    )
            else:
                for k2 in range(KT // 2):
                    nc.tensor.matmul(
                        ps,
                        (w_sb[:, half, 2 * k2:2 * k2 + 2,
                              mh * 128:(mh + 1) * 128]),
                        (x_sb[:, 2 * k2:2 * k2 + 2, tsl]),
                        start=(k2 == 0),
                        stop=(k2 == KT // 2 - 1),
                        perf_mode=DR,
                    )
            if has_bias:
                nc.vector.tensor_scalar(
                    out=dstT[:, g, slab, tsl], in0=ps,
                    scalar1=bq_sb[:, m:m + 1], scalar2=None, op0=ALU.add,
                )
            else:
                nc.vector.tensor_copy(out=dstT[:, g, slab, tsl], in_=ps)

        # ---------------- Phase A: upfront projections ----------------
        # Just k(g0, nb0) + q(g0, nb0): enough for sblock 0's first t-tiles.
        with tc.tile_pool(name="pa_ps", bufs=3, space="PSUM") as ppool:
            for m in range(2):
                ps = ppool.tile([128, TB], F32, tag="pp")
                qk_group(ps, wk_sb, xk_sb, kT8, m, 0, False)
            for m in range(2):
                ps = ppool.tile([128, TB], F32, tag="pp")
                qk_group(ps, wq_sb, xq_sb, qT8, m, 0, True)

        # ---------------- Phase B ----------------
        with tc.tile_pool(name="sc3p", bufs=1, space="PSUM") as sc3p, \
             tc.tile_pool(name="sc2p", bufs=1, space="PSUM") as sc2p, \
             tc.tile_pool(name="accp", bufs=1, space="PSUM") as accp, \
             tc.tile_pool(name="flxp", bufs=1, space="PSUM") as flxp, \
             tc.tile_pool(name="exs", bufs=EXBUFS) as expool, \
             tc.tile_pool(name="aon", bufs=2) as aopool, \
             tc.tile_pool(name="rcs", bufs=2) as rcpool, \
             tc.tile_pool(name="osg", bufs=2) as ospool:
            sc3 = sc3p.tile([128, 3, 512], F32, name="sc3")
            sc2 = sc2p.tile([128, 2, 512], F32, name="sc2")
            acc = accp.tile([128, 4, D + 1], F32, name="acc")
            flex = [
                flxp.tile([128, 512], F32, name="flexA"),
                flxp.tile([128, 512], F32, name="flexB"),
            ]
            fpar = [0]  # flex parity counter

            def next_flex():
                f = flex[fpar[0] % 2]
                fpar[0] += 1
                return f

            ex_tiles = {}
            aon_tiles = {}
            ostage = {}
            deferred = deque()

            def emit_paced(kind, args):
                if kind == "vproj":
                    cb, mt = args
                    f = next_flex()
                    ps = f[:, 0:128]
                    for kt in range(KT):
                        nc.tensor.matmul(
                            ps,
                            (xv_sb[:, kt, mt * 128:(mt + 1) * 128]),
                            (wv_sb[:, kt, cb * 128:(cb + 1) * 128]),
                            start=(kt == 0),
                            stop=(kt == KT - 1),
                        )
                    nc.vector.tensor_add(
                        out=vaug[:, mt, 2 * cb:2 * cb + 2, 0:D],
                        in0=ps.rearrange("p (h d) -> p h d", d=D),
                        in1=bv_sb[:, cb * 128:(cb + 1) * 128].rearrange(
                            "p (h d) -> p h d", d=D
                        ),
                    )
                elif kind == "qproj":
                    m, nb = args
                    qk_group(next_flex(), wq_sb, xq_sb, qT8, m, nb, True)
                else:  # kproj
                    m, nb = args
                    qk_group(next_flex(), wk_sb, xk_sb, kT8, m, nb, False)

            def do_transp(q, cb, aon, sb):
                if F32T:
                    tp = next_flex()[:, 0:128]
                    nc.tensor.transpose(
                        tp, (aon[:, q, cb * 2:(cb + 1) * 2, :]), (id32_sb)
                    )
                else:
                    tp = next_flex()[:, 0:64].bitcast(BF16)
                    nc.tensor.transpose(
                        tp, (aon[:, q, cb * 2:(cb + 1) * 2, :]), (id_sb)
                    )
                nc.vector.tensor_copy(
                    out=aoT[:, cb,
                            sb * 512 + q * 128:sb * 512 + (q + 1) * 128],
                    in_=tp,
                )

            def emit_attnv(ci, gts):
                ex = ex_tiles.pop(ci)
                for j, gt in enumerate(gts):
                    u, tt = gt // NTT, gt % NTT
                    sb, h = u // NHC, u % NHC
                    for q in range(4):
                        nc.tensor.matmul(
                            acc[:, q, :],
                            (ex[:, j, q * 128:(q + 1) * 128]),
                            (vaug[:, tt, h, :]),
                            start=(tt == 0 and q == 0),
                            stop=(tt == NTT - 1 and q == 3),
                            skip_group_check=True,
                        )
                    if tt == NTT - 1:
                        finish_unit(u)

            def finish_unit(u):
                sb, h = u // NHC, u % NHC
                rc = rcpool.tile([128, 4, 1], F32, tag="rc", name="rc")
                nc.vector.reciprocal(out=rc, in_=acc[:, :, D:D + 1])
                aon = aon_tiles[sb % 2]
                for q in range(4):
                    nc.vector.tensor_scalar(
                        out=aon[:, q, h, :],
                        in0=acc[:, q, 0:D],
                        scalar1=rc[:, q, :],
                        scalar2=None,
                        op0=ALU.mult,
                    )
                if h % 2 == 1 and h != NHC - 1:
                    cb = h // 2
                    for q in range(4):
                        def t_transp(q=q, cb=cb, aon=aon, sb=sb):
                            do_transp(q, cb, aon, sb)
                        deferred.append(t_transp)
                if h == NHC - 1:
                    schedule_outproj(sb, aon)

            def schedule_outproj(sb, aon):
                for mt in range(4):
                    t0 = sb * 512 + mt * 128

                    def t_transp3(q=mt, aon=aon, sb=sb):
                        do_transp(q, NHC // 2 - 1, aon, sb)
                    deferred.append(t_transp3)
                    for ep in range(E // 256):
                        def t_oproj(mt=mt, ep=ep, t0=t0):
                            if ep == 0:
                                ostage[mt % 2] = ospool.tile(
                                    [128, E], F32, tag="os", name="os"
                                )
                            op = next_flex()[:, 0:256]
                            for half in range(2):
                                ec = ep * 2 + half
                                for kc in range(NM):
                                    nc.tensor.matmul(
                                        op[:, half * 128:(half + 1) * 128],
                                        (aoT[:, kc, t0:t0 + 128]),
                                        (wo_sb[:, kc, ec * 128:(ec + 1) * 128]),
                                        start=(kc == 0 and half == 0),
                                        stop=(kc == NM - 1 and half == 1),
                                        skip_group_check=True,
                                    )
                            nc.vector.tensor_add(
                                out=ostage[mt % 2][:, ep * 256:(ep + 1) * 256],
                                in0=op,
                                in1=bo_sb[:, ep * 256:(ep + 1) * 256],
                            )
                            if ep % 2 == 1:
                                eh = ep // 2
                                os_t = ostage[mt % 2]

                                def t_dma(os_t=os_t, eh=eh, t0=t0):
                                    nc.sync.dma_start(
                                        out=out[t0:t0 + 128,
                                                eh * 512:(eh + 1) * 512],
                                        in_=os_t[:, eh * 512:(eh + 1) * 512],
                                    )
                                deferred.append(t_dma)
                        deferred.append(t_oproj)

            pending = deque()
            v_emitted = [False] * 4
            v_groups_left = [NTT] * 4

            def attnv_ready(ci):
                return all(
                    v_emitted[(gt // NTT % NHC) // 2] for gt in chunks[ci]
                )

            for ci, gts in enumerate(chunks):
                sc = sc3 if len(gts) == 3 else sc2
                for j, gt in enumerate(gts):
                    u, tt = gt // NTT, gt % NTT
                    sb, h = u // NHC, u % NHC
                    if h == 0 and tt == 0:
                        aon_tiles[sb % 2] = aopool.tile(
                            [128, 4, NHC, D], F32 if F32T else BF16,
                            tag="aon", name="aon"
                        )
                    g, hb = h // 4, h % 4
                    p0 = hb * 32
                    if NODR_SCORES or QK16:
                        for slab in range(2):
                            nc.tensor.matmul(
                                sc[:, j, :],
                                (kT8[p0:p0 + 32, g, slab,
                                     tt * 128:(tt + 1) * 128]),
                                (qT8[p0:p0 + 32, g, slab,
                                     sb * 512:(sb + 1) * 512]),
                                start=(slab == 0),
                                stop=(slab == 1),
                                tile_position=(p0, 0),
                            )
                    else:
                        nc.tensor.matmul(
                            sc[:, j, :],
                            (kT8[p0:p0 + 32, g, :, tt * 128:(tt + 1) * 128]),
                            (qT8[p0:p0 + 32, g, :, sb * 512:(sb + 1) * 512]),
                            start=True,
                            stop=True,
                            perf_mode=DR,
                            tile_position=(p0, 0),
                        )
                n = len(gts)
                ex = expool.tile([128, 3, 512], BF16, tag="ex", name="ex")
                nc.scalar.activation(
                    out=ex[:, 0:n, :], in_=sc[:, 0:n, :], func=AF.Exp, scale=0.125
                )
                ex_tiles[ci] = ex
                # paced projection work assigned to this chunk
                for kind, args in emit_at[ci]:
                    emit_paced(kind, args)
                    if kind == "vproj":
                        cb = args[0]
                        v_groups_left[cb] -= 1
                        if v_groups_left[cb] == 0:
                            v_emitted[cb] = True
                # attn@v for pending chunks whose v-deps are emitted
                # (cap catch-up at 2 chunks per step to avoid PE bursts)
                if ci > 0:
                    pending.append(ci - 1)
                drained = 0
                while pending and attnv_ready(pending[0]) and (
                    drained < 2 or ci - pending[0] >= EXBUFS - 3
                ):
                    emit_attnv(pending[0], chunks[pending[0]])
                    pending.popleft()
                    drained += 1
                if pending:
                    assert ci - pending[0] < EXBUFS - 1, (
                        f"ex ring overrun at chunk {ci}: pending {pending[0]}"
                    )
                for _ in range(min(3, len(deferred))):
                    deferred.popleft()()
            if DBG:
                nc.sync.dma_start(out=d_qT, in_=qT8)
                nc.sync.dma_start(out=d_kT, in_=kT8)
                nc.sync.dma_start(out=d_vaug, in_=vaug)
            pending.append(NCH - 1)
            while pending:
                emit_attnv(pending[0], chunks[pending[0]])
                pending.popleft()
            while deferred:
                deferred.popleft()()
            if DBG:
                nc.sync.dma_start(out=d_aoT, in_=aoT)

    nc.compile()
    return nc


_PROG = {}


def _get_prog(num_devices=8):
    if num_devices not in _PROG:
        _PROG[num_devices] = build_program(num_devices)
    return _PROG[num_devices]


FP8NP = ml_dtypes.float8_e4m3
BF16NP = ml_dtypes.bfloat16
QKNP = BF16NP if QK16 else FP8NP


def _tile_x(x2d, dt):
    # [S, E] -> [128, KT, S]; element (p, k, t) = x2d[t, k*128+p]
    xt = np.ascontiguousarray(x2d.T)
    return np.ascontiguousarray(
        xt.reshape(KT, 128, S).transpose(1, 0, 2).astype(dt)
    )


def _tile_w(weff_cols, dt):
    # weff_cols [E, CH] -> [128, KT, CH]; (p, k, c) = weff_cols[k*128+p, c]
    return np.ascontiguousarray(
        weff_cols.reshape(KT, 128, CH).transpose(1, 0, 2).astype(dt)
    )


def _tile_w_half(weff_cols, dt):
    # [E, CH] -> [128, 2, KT, CH//2]; (p, half, k, j) = W[k*128+p, half*256+j]
    wt = weff_cols.reshape(KT, 128, CH).transpose(1, 0, 2)  # [128, KT, CH]
    return np.ascontiguousarray(
        wt.reshape(128, KT, 2, CH // 2).transpose(0, 2, 1, 3).astype(dt)
    )


def _qk_perm():
    # channel permutation: m-tile m=(g,slab), partition p=(hb*32+r) holds
    # local channel (g*4+hb)*64 + slab*32 + r
    j = np.arange(CH)
    m, p = j // 128, j % 128
    g, slab = m // 2, m % 2
    hb, r = p // 32, p % 32
    return (g * 4 + hb) * 64 + slab * 32 + r


def prep_in_maps(x_q, x_k, x_v, Wq, bq, Aq, Bq, Wk, bk, Wv, bv, Av, Bv, Wo, bo):
    x_q = np.asarray(x_q, np.float32)
    x_k = np.asarray(x_k, np.float32)
    x_v = np.asarray(x_v, np.float32)
    scaling = 2.0  # lora_alpha / r
    wq_eff = (np.asarray(Wq).T + (np.asarray(Aq) @ np.asarray(Bq)) * scaling).astype(
        np.float32
    )
    wv_eff = (np.asarray(Wv).T + (np.asarray(Av) @ np.asarray(Bv)) * scaling).astype(
        np.float32
    )
    wk_T = np.asarray(Wk).T.astype(np.float32)  # bk dropped (softmax-invariant)
    bq = np.asarray(bq, np.float32)
    bv = np.asarray(bv, np.float32)
    bo = np.asarray(bo, np.float32)
    woT = np.ascontiguousarray(np.asarray(Wo).T.astype(np.float32))
    perm = _qk_perm()
    ident = np.eye(128, dtype=BF16NP)

    nbatch = x_q.shape[1]
    in_maps = []
    for c in range(2 * nbatch):
        b = c // 2
        hg = c % 2
        ch0 = hg * CH
        in_maps.append({
            "xq8": _tile_x(x_q[:, b, :], QKNP),
            "xk8": _tile_x(x_k[:, b, :], QKNP),
            "xv16": _tile_x(x_v[:, b, :], BF16NP),
            "wq8": _tile_w_half(wq_eff[:, ch0 + perm], QKNP),
            "wk8": _tile_w_half(wk_T[:, ch0 + perm], QKNP),
            "wv16": _tile_w(wv_eff[:, ch0:ch0 + CH], BF16NP),
            "wo16": np.ascontiguousarray(
                woT[ch0:ch0 + CH, :].reshape(NM, 128, E).transpose(1, 0, 2)
            ).astype(BF16NP),
            "bq": np.ascontiguousarray(bq[ch0 + perm].reshape(NM, 128).T),
            "bv16": np.ascontiguousarray(
                np.broadcast_to(bv[ch0:ch0 + CH], (128, CH))
            ).astype(BF16NP),
            "bo16": (
                np.ascontiguousarray(np.broadcast_to(bo, (128, E))).astype(BF16NP)
                if hg == 0
                else np.zeros((128, E), BF16NP)
            ),
            "ident": ident,
        })
    return in_maps


def gather_out(results, nbatch):
    return np.stack(
        [
            np.asarray(results[2 * b]["out"], np.float32)
            + np.asarray(results[2 * b + 1]["out"], np.float32)
            for b in range(nbatch)
        ],
        axis=1,
    )


def kernel(**inputs):
    nc = _get_prog(NCORES)
    in_maps = prep_in_maps(**inputs)
    res = run_bass_kernel_spmd(nc, in_maps, core_ids=list(range(NCORES)))
    return gather_out(res.results, B)


# revision 38
# speedup vs baseline: 1.5144x; 1.0344x over previous
"""LoRA multi-head attention kernel for 8 Trainium2 NeuronCores (v3).

Math: q = x_q@(Wq.T + Aq@Bq*2) + bq ; k = x_k@Wk.T (bk dropped: its score
term is constant over keys -> softmax-invariant) ; v = x_v@(Wv.T + Av@Bv*2)
+ bv ; 16-head attention, D=64; out = attn_out @ Wo.T + bo.

Sharding: core c = (batch b=c//2, head-group hg=c%2) -> 8 heads, 512
channels. LoRA merged on host (exact). Each core computes a partial output
(its 512 channels through Wo); host sums the two partials per batch.

Engine plan (TimelineSim cost model): ACT exp over the 8x2048x2048 score
tiles is the bottleneck (~0.83 ns/elem). Everything else is shaped to keep
ACT 100% busy:
  - scores in fp8e4 + DoubleRow ([32, 2-slab] layout), q/k projections in
    fp8e4 + DoubleRow (2x128 contraction per matmul).
  - attn@v in bf16 with OUTPUT [s-chunk=128 part, d+1]: full 128 output
    partitions per streamed column; softmax denominator rides in column 64.
  - per-head normalize via per-partition tensor_scalar; PE-transpose
    normalized [s, ch] -> aoT [ch, s]; bf16 output projection.
  - PSUM banks (groups zero a full 2 KiB bank on start -> one open group
    per bank): sc ring 3+2 banks (alternating chunk sizes), 1 acc bank
    (one open group per unit), 2 flex banks shared by paced projection /
    transpose / outproj groups in strict A/B alternation.
  - phase B starts right after k(g0)+q(g0,nb0); the remaining projections
    (q rest, k/q g1, all of v in head-pair-major order) are paced into the
    chunk stream ahead of static deadlines so ACT never waits.
"""

import os
import sys

import numpy as np

sys.path.insert(0, "/opt/trn_rl_repo")

NODR_PROJ = bool(int(os.environ.get("NODR_PROJ", "0")))
NODR_SCORES = bool(int(os.environ.get("NODR_SCORES", "0")))
F32T = bool(int(os.environ.get("F32T", "0")))
DBG = bool(int(os.environ.get("DBG", "0")))
QK16 = bool(int(os.environ.get("QK16", "0")))
DRAIN = int(os.environ.get("DRAIN", "1"))

from collections import deque  # noqa: E402
from contextlib import ExitStack  # noqa: E402

import ml_dtypes  # noqa: E402

import concourse.bass as bass  # noqa: E402
import concourse.tile as tile  # noqa: E402
from concourse import bacc, mybir  # noqa: E402
from concourse.bass_utils import run_bass_kernel_spmd  # noqa: E402

F32 = mybir.dt.float32
BF16 = mybir.dt.bfloat16
FP8 = mybir.dt.float8e4
AF = mybir.ActivationFunctionType
ALU = mybir.AluOpType
DR = mybir.MatmulPerfMode.DoubleRow
QKDT = BF16 if QK16 else FP8

E = 1024
D = 64
NHC = 8            # heads per core
CH = NHC * D       # 512 output channels per core
KT = E // 128      # 8 k-tiles over the E contraction
NCORES = 8
B = 4
S = 2048
NTT = S // 128     # 16 t tiles
NSB = S // 512     # 4 s blocks
NM = CH // 128     # 4 ch tiles per projection
TB = 512           # projection token block
NTB = S // TB
NU = NSB * NHC     # 32 (sblock, head) units
NGT = NU * NTT     # 512 score tiles of [128, 512]
EXBUFS = int(os.environ.get("EXBUFS", "14"))


def _chunk_list():
    """Alternating 3/2-tile chunks over the NGT score tiles."""
    chunks = []
    gt = 0
    size = 3
    while gt < NGT:
        n = min(size, NGT - gt)
        chunks.append(list(range(gt, gt + n)))
        gt += n
        size = 5 - size
    return chunks


def build_program(num_devices=8):
    nc = bacc.Bacc(
        "TRN2", target_bir_lowering=False, debug=False, num_devices=num_devices
    )

    def dram(name, shape, dt=F32, out=False):
        kind = "ExternalOutput" if out else "ExternalInput"
        return nc.dram_tensor(name, shape, dt, kind=kind).ap()

    xq8 = dram("xq8", [128, KT, S], QKDT)
    xk8 = dram("xk8", [128, KT, S], QKDT)
    xv16 = dram("xv16", [128, KT, S], BF16)
    wq8 = dram("wq8", [128, 2, KT, CH // 2], QKDT)
    wk8 = dram("wk8", [128, 2, KT, CH // 2], QKDT)
    wv16 = dram("wv16", [128, KT, CH], BF16)
    wo16 = dram("wo16", [128, NM, E], BF16)
    bq = dram("bq", [128, NM])
    bv16 = dram("bv16", [128, CH], BF16)
    bo16 = dram("bo16", [128, E], BF16)
    ident = dram("ident", [128, 128], BF16)
    out = dram("out", [S, E], out=True)
    if DBG:
        d_qT = dram("d_qT", [128, 2, 2, S], FP8, out=True)
        d_kT = dram("d_kT", [128, 2, 2, S], FP8, out=True)
        d_vaug = dram("d_vaug", [128, NTT, NHC, D + 1], BF16, out=True)
        d_aoT = dram("d_aoT", [128, NM, S], BF16, out=True)

    chunks = _chunk_list()
    NCH = len(chunks)
    # chunk index that finishes each unit (contains gt = u*16+15)
    unit_end_chunk = {}
    for ci, gts in enumerate(chunks):
        for gt in gts:
            if gt % NTT == NTT - 1:
                unit_end_chunk[gt // NTT] = ci

    # ---- static pacing schedule for leftover projection work ----
    # paced items (emitted through the 2 flex psum banks):
    #   v groups:  (cb, mt) for cb in 0..3, mt in 0..15   deadline per cb
    #   kq g1:     (proj, m, nb) for m in 2,3             deadline c(u4)
    #   q g0 rest: (q, m in 0..1, nb in 1..3)             deadline per sb
    def c_of_u(u):
        return unit_end_chunk.get(u, NCH - 1)

    paced = []  # (deadline_chunk, kind, args)
    for nb in range(1, NTB):
        dl = max(0, c_of_u(nb * NHC) - NHC * 3)  # before sblock nb begins
        for m in range(2):
            paced.append((dl, "qproj", (m, nb)))
    # k g0 for tokens nb>=1: needed by sblock-0 scores at t-tile 4*nb
    for nb in range(1, NTB):
        first_gt = 4 * nb
        dl = max(0, first_gt // 5 * 2 + (1 if first_gt % 5 >= 3 else 0) - 1)
        for m in range(2):
            paced.append((dl, "kproj", (m, nb)))
    dl_g1 = max(0, c_of_u(3) - 4)  # before first g1 head (u4) scores
    for m in range(2, NM):
        for nb in range(NTB):
            paced.append((dl_g1, "kproj", (m, nb)))
            paced.append((dl_g1, "qproj", (m, nb)))
    # v deadlines: cb's first consumer unit (2*cb) starts pinning ex tiles
    # at the end of unit 2*cb-1; all of cb must be emitted within the ex
    # ring window from there.
    for cb in range(4):
        start_c = 0 if cb == 0 else c_of_u(2 * cb - 1)
        dl = max(2, start_c + EXBUFS - 4)
        for mt in range(NTT):
            paced.append((dl, "vproj", (cb, mt)))
    paced.sort(key=lambda t: t[0])
    # assign each paced item an emission chunk: spread from 0 to deadline
    # greedily in deadline order, at most MAXP per chunk
    MAXP = int(os.environ.get("MAXP", "2"))
    slots = [0] * NCH
    emit_at = [[] for _ in range(NCH)]
    for dl, kind, args in paced:
        c = 0
        while slots[c] >= MAXP and c < dl:
            c += 1
        slots[c] += 1
        emit_at[c].append((kind, args))

    with tile.TileContext(nc) as tc, ExitStack() as top:
        persist = top.enter_context(tc.tile_pool(name="persist", bufs=1))
        qT8 = persist.tile([128, 2, 2, S], QKDT)     # [hb*32+r, g, slab, s]
        kT8 = persist.tile([128, 2, 2, S], QKDT)
        vaug = persist.tile([128, NTT, NHC, D + 1], BF16)
        aoT = persist.tile([128, NM, S], BF16)       # [ch%128, ch//128, tok]
        wq_sb = persist.tile([128, 2, KT, CH // 2], QKDT)
        wk_sb = persist.tile([128, 2, KT, CH // 2], QKDT)
        wv_sb = persist.tile([128, KT, CH], BF16)
        wo_sb = persist.tile([128, NM, E], BF16)
        bq_sb = persist.tile([128, NM], F32)
        bv_sb = persist.tile([128, CH], BF16)
        bo_sb = persist.tile([128, E], BF16)
        id_sb = persist.tile([128, 128], BF16)
        id32_sb = persist.tile([128, 128], F32)
        xq_sb = persist.tile([128, KT, S], FP8)
        xk_sb = persist.tile([128, KT, S], FP8)
        xv_sb = persist.tile([128, KT, S], BF16)

        # DMA order = earliest-need order (halved w tensors keep each DMA
        # contiguous at full bus width).
        nc.sync.dma_start(out=wk_sb[:, 0], in_=wk8[:, 0])
        nc.sync.dma_start(out=xk_sb[:, :, 0:TB], in_=xk8[:, :, 0:TB])
        nc.sync.dma_start(out=wq_sb[:, 0], in_=wq8[:, 0])
        nc.sync.dma_start(out=xq_sb[:, :, 0:TB], in_=xq8[:, :, 0:TB])
        nc.sync.dma_start(out=bq_sb, in_=bq)
        for i in range(1, NTB):
            sl = slice(i * TB, (i + 1) * TB)
            nc.sync.dma_start(out=xk_sb[:, :, sl], in_=xk8[:, :, sl])
        nc.sync.dma_start(out=wv_sb, in_=wv16)
        nc.sync.dma_start(out=bv_sb, in_=bv16)
        for i in range(NTB):
            sl = slice(i * TB, (i + 1) * TB)
            nc.sync.dma_start(out=xv_sb[:, :, sl], in_=xv16[:, :, sl])
        nc.sync.dma_start(out=id_sb, in_=ident)
        nc.sync.dma_start(out=wk_sb[:, 1], in_=wk8[:, 1])
        nc.sync.dma_start(out=wq_sb[:, 1], in_=wq8[:, 1])
        for i in range(1, NTB):
            sl = slice(i * TB, (i + 1) * TB)
            nc.sync.dma_start(out=xq_sb[:, :, sl], in_=xq8[:, :, sl])
        nc.sync.dma_start(out=wo_sb, in_=wo16)
        nc.sync.dma_start(out=bo_sb, in_=bo16)
        nc.vector.memset(vaug[:, :, :, D:D + 1], 1.0)
        if F32T:
            nc.vector.tensor_copy(out=id32_sb, in_=id_sb)

        def qk_group(ps, w_sb, x_sb, dstT, m, nb, has_bias):
            g, slab = m // 2, m % 2
            half, mh = m // 2, m % 2
            tsl = slice(nb * TB, (nb + 1) * TB)
            if NODR_PROJ or QK16:
                for kt in range(KT):
                    nc.tensor.matmul(
                        ps,
                        (w_sb[:, half, kt, mh * 128:(mh + 1) * 128]),
                        (x_sb[:, kt, tsl]),
                        start=(kt == 0),
                        stop=(kt == KT - 1),
                    )
            else:
                for k2 in range(KT // 2):
                    nc.tensor.matmul(
                        ps,
                        (w_sb[:, half, 2 * k2:2 * k2 + 2,
                              mh * 128:(mh + 1) * 128]),
                        (x_sb[:, 2 * k2:2 * k2 + 2, tsl]),
                        start=(k2 == 0),
                        stop=(k2 == KT // 2 - 1),
                        perf_mode=DR,
                    )
            if has_bias:
                nc.vector.tensor_scalar(
                    out=dstT[:, g, slab, tsl], in0=ps,
                    scalar1=bq_sb[:, m:m + 1], scalar2=None, op0=ALU.add,
                )
            else:
                nc.vector.tensor_copy(out=dstT[:, g, slab, tsl], in_=ps)

        # ---------------- Phase A: upfront projections ----------------
        # Just k(g0, nb0) + q(g0, nb0): enough for sblock 0's first t-tiles.
        with tc.tile_pool(name="pa_ps", bufs=3, space="PSUM") as ppool:
            for m in range(2):
                ps = ppool.tile([128, TB], F32, tag="pp")
                qk_group(ps, wk_sb, xk_sb, kT8, m, 0, False)
            for m in range(2):
                ps = ppool.tile([128, TB], F32, tag="pp")
                qk_group(ps, wq_sb, xq_sb, qT8, m, 0, True)

        # ---------------- Phase B ----------------
        with tc.tile_pool(name="sc3p", bufs=1, space="PSUM") as sc3p, \
             tc.tile_pool(name="sc2p", bufs=1, space="PSUM") as sc2p, \
             tc.tile_pool(name="accp", bufs=1, space="PSUM") as accp, \
             tc.tile_pool(name="flxp", bufs=1, space="PSUM") as flxp, \
             tc.tile_pool(name="exs", bufs=EXBUFS) as expool, \
             tc.tile_pool(name="aon", bufs=2) as aopool, \
             tc.tile_pool(name="rcs", bufs=2) as rcpool, \
             tc.tile_pool(name="osg", bufs=2) as ospool:
            sc3 = sc3p.tile([128, 3, 512], F32, name="sc3")
            sc2 = sc2p.tile([128, 2, 512], F32, name="sc2")
            acc = accp.tile([128, 4, D + 1], F32, name="acc")
            acc_flat = acc.rearrange("p q x -> p (q x)")
            flex = [
                flxp.tile([128, 512], F32, name="flexA"),
                flxp.tile([128, 512], F32, name="flexB"),
            ]
            fpar = [0]  # flex parity counter
            tail_mode = [False]
            front_mode = [True]   # until first attn@v uses the acc bank

            def next_flex(big=False):
                if tail_mode[0]:
                    ring = [flex[0], flex[1], sc2[:, 0, :], sc2[:, 1, :],
                            acc_flat]
                elif front_mode[0]:
                    ring = [flex[0], flex[1], acc_flat]
                else:
                    ring = [flex[0], flex[1]]
                f = ring[fpar[0] % len(ring)]
                fpar[0] += 1
                while big and f.free_size() < 512:
                    f = ring[fpar[0] % len(ring)]
                    fpar[0] += 1
                return f

            ex_tiles = {}
            aon_tiles = {}
            ostage = {}
            deferred = deque()

            def emit_paced(kind, args):
                if kind == "vproj":
                    cb, mt = args
                    f = next_flex()
                    ps = f[:, 0:128]
                    for kt in range(KT):
                        nc.tensor.matmul(
                            ps,
                            (xv_sb[:, kt, mt * 128:(mt + 1) * 128]),
                            (wv_sb[:, kt, cb * 128:(cb + 1) * 128]),
                            start=(kt == 0),
                            stop=(kt == KT - 1),
                        )
                    nc.vector.tensor_add(
                        out=vaug[:, mt, 2 * cb:2 * cb + 2, 0:D],
                        in0=ps.rearrange("p (h d) -> p h d", d=D),
                        in1=bv_sb[:, cb * 128:(cb + 1) * 128].rearrange(
                            "p (h d) -> p h d", d=D
                        ),
                    )
                elif kind == "qproj":
                    m, nb = args
                    qk_group(next_flex(big=True), wq_sb, xq_sb, qT8, m, nb, True)
                else:  # kproj
                    m, nb = args
                    qk_group(next_flex(big=True), wk_sb, xk_sb, kT8, m, nb, False)

            def do_transp(q, cb, aon, sb):
                if F32T:
                    tp = next_flex()[:, 0:128]
                    nc.tensor.transpose(
                        tp, (aon[:, q, cb * 2:(cb + 1) * 2, :]), (id32_sb)
                    )
                else:
                    tp = next_flex()[:, 0:64].bitcast(BF16)
                    nc.tensor.transpose(
                        tp, (aon[:, q, cb * 2:(cb + 1) * 2, :]), (id_sb)
                    )
                nc.vector.tensor_copy(
                    out=aoT[:, cb,
                            sb * 512 + q * 128:sb * 512 + (q + 1) * 128],
                    in_=tp,
                )

            def emit_attnv(ci, gts):
                ex = ex_tiles.pop(ci)
                for j, gt in enumerate(gts):
                    u, tt = gt // NTT, gt % NTT
                    sb, h = u // NHC, u % NHC
                    for q in range(4):
                        nc.tensor.matmul(
                            acc[:, q, :],
                            (ex[:, j, q * 128:(q + 1) * 128]),
                            (vaug[:, tt, h, :]),
                            start=(tt == 0 and q == 0),
                            stop=(tt == NTT - 1 and q == 3),
                            skip_group_check=True,
                        )
                    if tt == NTT - 1:
                        finish_unit(u)

            def finish_unit(u):
                sb, h = u // NHC, u % NHC
                rc = rcpool.tile([128, 4, 1], F32, tag="rc", name="rc")
                nc.vector.reciprocal(out=rc, in_=acc[:, :, D:D + 1])
                aon = aon_tiles[sb % 2]
                for q in range(4):
                    nc.vector.tensor_scalar(
                        out=aon[:, q, h, :],
                        in0=acc[:, q, 0:D],
                        scalar1=rc[:, q, :],
                        scalar2=None,
                        op0=ALU.mult,
                    )
                if h % 2 == 1 and h != NHC - 1:
                    cb = h // 2
                    for q in range(4):
                        def t_transp(q=q, cb=cb, aon=aon, sb=sb):
                            do_transp(q, cb, aon, sb)
                        deferred.append(t_transp)
                if h == NHC - 1:
                    schedule_outproj(sb, aon)

            def schedule_outproj(sb, aon):
                for mt in range(4):
                    t0 = sb * 512 + mt * 128

                    def t_transp3(q=mt, aon=aon, sb=sb):
                        do_transp(q, NHC // 2 - 1, aon, sb)
                    deferred.append(t_transp3)
                    for ep in range(E // 256):
                        def t_oproj(mt=mt, ep=ep, t0=t0):
                            if ep == 0:
                                ostage[mt % 2] = ospool.tile(
                                    [128, E], F32, tag="os", name="os"
                                )
                            op = next_flex()[:, 0:256]
                            for half in range(2):
                                ec = ep * 2 + half
                                for kc in range(NM):
                                    nc.tensor.matmul(
                                        op[:, half * 128:(half + 1) * 128],
                                        (aoT[:, kc, t0:t0 + 128]),
                                        (wo_sb[:, kc, ec * 128:(ec + 1) * 128]),
                                        start=(kc == 0 and half == 0),
                                        stop=(kc == NM - 1 and half == 1),
                                        skip_group_check=True,
                                    )
                            nc.vector.tensor_add(
                                out=ostage[mt % 2][:, ep * 256:(ep + 1) * 256],
                                in0=op,
                                in1=bo_sb[:, ep * 256:(ep + 1) * 256],
                            )
                            if ep % 2 == 1:
                                eh = ep // 2
                                os_t = ostage[mt % 2]

                                def t_dma(os_t=os_t, eh=eh, t0=t0):
                                    nc.sync.dma_start(
                                        out=out[t0:t0 + 128,
                                                eh * 512:(eh + 1) * 512],
                                        in_=os_t[:, eh * 512:(eh + 1) * 512],
                                    )
                                deferred.append(t_dma)
                        deferred.append(t_oproj)

            pending = deque()
            v_emitted = set()

            def attnv_ready(ci):
                return all(
                    ((gt // NTT % NHC) // 2, gt % NTT) in v_emitted
                    for gt in chunks[ci]
                )

            for ci, gts in enumerate(chunks):
                sc = sc3 if len(gts) == 3 else sc2
                for j, gt in enumerate(gts):
                    u, tt = gt // NTT, gt % NTT
                    sb, h = u // NHC, u % NHC
                    if h == 0 and tt == 0:
                        aon_tiles[sb % 2] = aopool.tile(
                            [128, 4, NHC, D], F32 if F32T else BF16,
                            tag="aon", name="aon"
                        )
                    g, hb = h // 4, h % 4
                    p0 = hb * 32
                    if NODR_SCORES or QK16:
                        for slab in range(2):
                            nc.tensor.matmul(
                                sc[:, j, :],
                                (kT8[p0:p0 + 32, g, slab,
                                     tt * 128:(tt + 1) * 128]),
                                (qT8[p0:p0 + 32, g, slab,
                                     sb * 512:(sb + 1) * 512]),
                                start=(slab == 0),
                                stop=(slab == 1),
                                tile_position=(p0, 0),
                            )
                    else:
                        nc.tensor.matmul(
                            sc[:, j, :],
                            (kT8[p0:p0 + 32, g, :, tt * 128:(tt + 1) * 128]),
                            (qT8[p0:p0 + 32, g, :, sb * 512:(sb + 1) * 512]),
                            start=True,
                            stop=True,
                            perf_mode=DR,
                            tile_position=(p0, 0),
                        )
                n = len(gts)
                ex = expool.tile([128, 3, 512], BF16, tag="ex", name="ex")
                nc.scalar.activation(
                    out=ex[:, 0:n, :], in_=sc[:, 0:n, :], func=AF.Exp, scale=0.125
                )
                ex_tiles[ci] = ex
                # attn@v for pending chunks whose v-deps are emitted
                # (cap catch-up at 2 chunks per step to avoid PE bursts)
                if ci > 0:
                    pending.append(ci - 1)
                drained = 0
                while pending and attnv_ready(pending[0]) and (
                    drained < 2 or ci - pending[0] >= EXBUFS - 3
                ):
                    front_mode[0] = False
                    emit_attnv(pending[0], chunks[pending[0]])
                    pending.popleft()
                    drained += 1
                # paced projection work assigned to this chunk
                for kind, args in emit_at[ci]:
                    emit_paced(kind, args)
                    if kind == "vproj":
                        v_emitted.add(args)
                if pending:
                    assert ci - pending[0] < EXBUFS - 1, (
                        f"ex ring overrun at chunk {ci}: pending {pending[0]}"
                    )
                k = max(0, DRAIN - len(emit_at[ci]))
                for _ in range(min(k, len(deferred))):
                    deferred.popleft()()
            if DBG:
                nc.sync.dma_start(out=d_qT, in_=qT8)
                nc.sync.dma_start(out=d_kT, in_=kT8)
                nc.sync.dma_start(out=d_vaug, in_=vaug)
            tail_mode[0] = True
            pending.append(NCH - 1)
            while pending:
                emit_attnv(pending[0], chunks[pending[0]])
                pending.popleft()
            while deferred:
                deferred.popleft()()
            if DBG:
                nc.sync.dma_start(out=d_aoT, in_=aoT)

    nc.compile()
    return nc


_PROG = {}


def _get_prog(num_devices=8):
    if num_devices not in _PROG:
        _PROG[num_devices] = build_program(num_devices)
    return _PROG[num_devices]


FP8NP = ml_dtypes.float8_e4m3
BF16NP = ml_dtypes.bfloat16
QKNP = BF16NP if QK16 else FP8NP


def _tile_x(x2d, dt):
    # [S, E] -> [128, KT, S]; element (p, k, t) = x2d[t, k*128+p]
    xt = np.ascontiguousarray(x2d.T)
    return np.ascontiguousarray(
        xt.reshape(KT, 128, S).transpose(1, 0, 2).astype(dt)
    )


def _tile_w(weff_cols, dt):
    # weff_cols [E, CH] -> [128, KT, CH]; (p, k, c) = weff_cols[k*128+p, c]
    return np.ascontiguousarray(
        weff_cols.reshape(KT, 128, CH).transpose(1, 0, 2).astype(dt)
    )


def _tile_w_half(weff_cols, dt):
    # [E, CH] -> [128, 2, KT, CH//2]; (p, half, k, j) = W[k*128+p, half*256+j]
    wt = weff_cols.reshape(KT, 128, CH).transpose(1, 0, 2)  # [128, KT, CH]
    return np.ascontiguousarray(
        wt.reshape(128, KT, 2, CH // 2).transpose(0, 2, 1, 3).astype(dt)
    )


def _qk_perm():
    # channel permutation: m-tile m=(g,slab), partition p=(hb*32+r) holds
    # local channel (g*4+hb)*64 + slab*32 + r
    j = np.arange(CH)
    m, p = j // 128, j % 128
    g, slab = m // 2, m % 2
    hb, r = p // 32, p % 32
    return (g * 4 + hb) * 64 + slab * 32 + r


def prep_in_maps(x_q, x_k, x_v, Wq, bq, Aq, Bq, Wk, bk, Wv, bv, Av, Bv, Wo, bo):
    x_q = np.asarray(x_q, np.float32)
    x_k = np.asarray(x_k, np.float32)
    x_v = np.asarray(x_v, np.float32)
    scaling = 2.0  # lora_alpha / r
    wq_eff = (np.asarray(Wq).T + (np.asarray(Aq) @ np.asarray(Bq)) * scaling).astype(
        np.float32
    )
    wv_eff = (np.asarray(Wv).T + (np.asarray(Av) @ np.asarray(Bv)) * scaling).astype(
        np.float32
    )
    wk_T = np.asarray(Wk).T.astype(np.float32)  # bk dropped (softmax-invariant)
    bq = np.asarray(bq, np.float32)
    bv = np.asarray(bv, np.float32)
    bo = np.asarray(bo, np.float32)
    woT = np.ascontiguousarray(np.asarray(Wo).T.astype(np.float32))
    perm = _qk_perm()
    ident = np.eye(128, dtype=BF16NP)

    nbatch = x_q.shape[1]
    in_maps = []
    for c in range(2 * nbatch):
        b = c // 2
        hg = c % 2
        ch0 = hg * CH
        in_maps.append({
            "xq8": _tile_x(x_q[:, b, :], QKNP),
            "xk8": _tile_x(x_k[:, b, :], QKNP),
            "xv16": _tile_x(x_v[:, b, :], BF16NP),
            "wq8": _tile_w_half(wq_eff[:, ch0 + perm], QKNP),
            "wk8": _tile_w_half(wk_T[:, ch0 + perm], QKNP),
            "wv16": _tile_w(wv_eff[:, ch0:ch0 + CH], BF16NP),
            "wo16": np.ascontiguousarray(
                woT[ch0:ch0 + CH, :].reshape(NM, 128, E).transpose(1, 0, 2)
            ).astype(BF16NP),
            "bq": np.ascontiguousarray(bq[ch0 + perm].reshape(NM, 128).T),
            "bv16": np.ascontiguousarray(
                np.broadcast_to(bv[ch0:ch0 + CH], (128, CH))
            ).astype(BF16NP),
            "bo16": (
                np.ascontiguousarray(np.broadcast_to(bo, (128, E))).astype(BF16NP)
                if hg == 0
                else np.zeros((128, E), BF16NP)
            ),
            "ident": ident,
        })
    return in_maps


def gather_out(results, nbatch):
    return np.stack(
        [
            np.asarray(results[2 * b]["out"], np.float32)
            + np.asarray(results[2 * b + 1]["out"], np.float32)
            for b in range(nbatch)
        ],
        axis=1,
    )


def kernel(**inputs):
    nc = _get_prog(NCORES)
    in_maps = prep_in_maps(**inputs)
    res = run_bass_kernel_spmd(nc, in_maps, core_ids=list(range(NCORES)))
    return gather_out(res.results, B)


# revision 44
# speedup vs baseline: 1.5283x; 1.0092x over previous
"""LoRA multi-head attention kernel for 8 Trainium2 NeuronCores (v3).

Math: q = x_q@(Wq.T + Aq@Bq*2) + bq ; k = x_k@Wk.T (bk dropped: its score
term is constant over keys -> softmax-invariant) ; v = x_v@(Wv.T + Av@Bv*2)
+ bv ; 16-head attention, D=64; out = attn_out @ Wo.T + bo.

Sharding: core c = (batch b=c//2, head-group hg=c%2) -> 8 heads, 512
channels. LoRA merged on host (exact). Each core computes a partial output
(its 512 channels through Wo); host sums the two partials per batch.

Engine plan (TimelineSim cost model): ACT exp over the 8x2048x2048 score
tiles is the bottleneck (~0.83 ns/elem). Everything else is shaped to keep
ACT 100% busy:
  - scores in fp8e4 + DoubleRow ([32, 2-slab] layout), q/k projections in
    fp8e4 + DoubleRow (2x128 contraction per matmul).
  - attn@v in bf16 with OUTPUT [s-chunk=128 part, d+1]: full 128 output
    partitions per streamed column; softmax denominator rides in column 64.
  - per-head normalize via per-partition tensor_scalar; PE-transpose
    normalized [s, ch] -> aoT [ch, s]; bf16 output projection.
  - PSUM banks (groups zero a full 2 KiB bank on start -> one open group
    per bank): sc ring 3+2 banks (alternating chunk sizes), 1 acc bank
    (one open group per unit), 2 flex banks shared by paced projection /
    transpose / outproj groups in strict A/B alternation.
  - phase B starts right after k(g0)+q(g0,nb0); the remaining projections
    (q rest, k/q g1, all of v in head-pair-major order) are paced into the
    chunk stream ahead of static deadlines so ACT never waits.
"""

import os
import sys

import numpy as np

sys.path.insert(0, "/opt/trn_rl_repo")

NODR_PROJ = bool(int(os.environ.get("NODR_PROJ", "0")))
NODR_SCORES = bool(int(os.environ.get("NODR_SCORES", "0")))
F32T = bool(int(os.environ.get("F32T", "0")))
DBG = bool(int(os.environ.get("DBG", "0")))
QK16 = bool(int(os.environ.get("QK16", "0")))
DRAIN = int(os.environ.get("DRAIN", "1"))
QKDRAIN_POOL = bool(int(os.environ.get("QKDRAIN_POOL", "0")))

from collections import deque  # noqa: E402
from contextlib import ExitStack  # noqa: E402

import ml_dtypes  # noqa: E402

import concourse.bass as bass  # noqa: E402
import concourse.tile as tile  # noqa: E402
from concourse import bacc, mybir  # noqa: E402
from concourse.bass_utils import run_bass_kernel_spmd  # noqa: E402

F32 = mybir.dt.float32
BF16 = mybir.dt.bfloat16
FP8 = mybir.dt.float8e4
AF = mybir.ActivationFunctionType
ALU = mybir.AluOpType
DR = mybir.MatmulPerfMode.DoubleRow
QKDT = BF16 if QK16 else FP8

E = 1024
D = 64
NHC = 8            # heads per core
CH = NHC * D       # 512 output channels per core
KT = E // 128      # 8 k-tiles over the E contraction
NCORES = 8
B = 4
S = 2048
NTT = S // 128     # 16 t tiles
NSB = S // 512     # 4 s blocks
NM = CH // 128     # 4 ch tiles per projection
TB = 512           # projection token block
NTB = S // TB
NU = NSB * NHC     # 32 (sblock, head) units
NGT = NU * NTT     # 512 score tiles of [128, 512]
EXBUFS = int(os.environ.get("EXBUFS", "18"))


def _chunk_list():
    """Alternating 3/2-tile chunks over the NGT score tiles."""
    chunks = []
    gt = 0
    size = 3
    while gt < NGT:
        n = min(size, NGT - gt)
        chunks.append(list(range(gt, gt + n)))
        gt += n
        size = 5 - size
    return chunks


def build_program(num_devices=8):
    nc = bacc.Bacc(
        "TRN2", target_bir_lowering=False, debug=False, num_devices=num_devices
    )

    def dram(name, shape, dt=F32, out=False):
        kind = "ExternalOutput" if out else "ExternalInput"
        return nc.dram_tensor(name, shape, dt, kind=kind).ap()

    xq8 = dram("xq8", [128, KT, S], QKDT)
    xk8 = dram("xk8", [128, KT, S], QKDT)
    xv16 = dram("xv16", [128, KT, S], BF16)
    wq8 = dram("wq8", [128, 2, KT, CH // 2], QKDT)
    wk8 = dram("wk8", [128, 2, KT, CH // 2], QKDT)
    wv16 = dram("wv16", [128, KT, CH], BF16)
    wo16 = dram("wo16", [128, NM, E], BF16)
    bq = dram("bq", [128, NM])
    bv16 = dram("bv16", [128, CH], BF16)
    bo16 = dram("bo16", [128, E], BF16)
    ident = dram("ident", [128, 128], BF16)
    out = dram("out", [S, E], out=True)
    if DBG:
        d_qT = dram("d_qT", [128, 2, 2, S], FP8, out=True)
        d_kT = dram("d_kT", [128, 2, 2, S], FP8, out=True)
        d_vaug = dram("d_vaug", [128, NTT, NHC, D + 1], BF16, out=True)
        d_aoT = dram("d_aoT", [128, NM, S], BF16, out=True)

    chunks = _chunk_list()
    NCH = len(chunks)
    # chunk index that finishes each unit (contains gt = u*16+15)
    unit_end_chunk = {}
    for ci, gts in enumerate(chunks):
        for gt in gts:
            if gt % NTT == NTT - 1:
                unit_end_chunk[gt // NTT] = ci

    # ---- static pacing schedule for leftover projection work ----
    # paced items (emitted through the 2 flex psum banks):
    #   v groups:  (cb, mt) for cb in 0..3, mt in 0..15   deadline per cb
    #   kq g1:     (proj, m, nb) for m in 2,3             deadline c(u4)
    #   q g0 rest: (q, m in 0..1, nb in 1..3)             deadline per sb
    def c_of_u(u):
        return unit_end_chunk.get(u, NCH - 1)

    paced = []  # (deadline_chunk, kind, args)
    for nb in range(1, NTB):
        dl = max(0, c_of_u(nb * NHC) - NHC * 3)  # before sblock nb begins
        for m in range(2):
            paced.append((dl, "qproj", (m, nb)))
    # k g0 for tokens nb>=1: needed by sblock-0 scores at t-tile 4*nb
    for nb in range(1, NTB):
        first_gt = 4 * nb
        dl = max(0, first_gt // 5 * 2 + (1 if first_gt % 5 >= 3 else 0) - 1)
        for m in range(2):
            paced.append((dl, "kproj", (m, nb)))
    dl_g1 = max(0, c_of_u(3) - 4)  # before first g1 head (u4) scores
    for m in range(2, NM):
        for nb in range(NTB):
            paced.append((dl_g1, "kproj", (m, nb)))
            paced.append((dl_g1, "qproj", (m, nb)))
    # v deadlines: cb's first consumer unit (2*cb) starts pinning ex tiles
    # at the end of unit 2*cb-1; all of cb must be emitted within the ex
    # ring window from there.
    for cb in range(4):
        start_c = 0 if cb == 0 else c_of_u(2 * cb - 1)
        dl = max(2, start_c + EXBUFS - 4)
        for mt in range(NTT):
            paced.append((dl, "vproj", (cb, mt)))
    paced.sort(key=lambda t: t[0])
    # assign each paced item an emission chunk: spread from 0 to deadline
    # greedily in deadline order, at most MAXP per chunk
    MAXP = int(os.environ.get("MAXP", "2"))
    slots = [0] * NCH
    emit_at = [[] for _ in range(NCH)]
    for dl, kind, args in paced:
        c = 0
        while slots[c] >= MAXP and c < dl:
            c += 1
        slots[c] += 1
        emit_at[c].append((kind, args))

    with tile.TileContext(nc) as tc, ExitStack() as top:
        persist = top.enter_context(tc.tile_pool(name="persist", bufs=1))
        qT8 = persist.tile([128, 2, 2, S], QKDT)     # [hb*32+r, g, slab, s]
        kT8 = persist.tile([128, 2, 2, S], QKDT)
        vaug = persist.tile([128, NTT, NHC, D + 1], BF16)
        aoT = persist.tile([128, NM, 2, 512], BF16)  # [ch, kc, sb%2, tok]
        wq_sb = persist.tile([128, 2, KT, CH // 2], QKDT)
        wk_sb = persist.tile([128, 2, KT, CH // 2], QKDT)
        wv_sb = persist.tile([128, KT, CH], BF16)
        wo_sb = persist.tile([128, NM, E], BF16)
        bq_sb = persist.tile([128, NM], F32)
        bv_sb = persist.tile([128, CH], BF16)
        bo_sb = persist.tile([128, E], BF16)
        id_sb = persist.tile([128, 128], BF16)
        id32_sb = persist.tile([128, 128], F32)
        xq_sb = persist.tile([128, KT, S], FP8)
        xk_sb = persist.tile([128, KT, S], FP8)
        xv_sb = persist.tile([128, KT, S], BF16)

        # DMA order = earliest-need order (halved w tensors keep each DMA
        # contiguous at full bus width).
        nc.sync.dma_start(out=wk_sb[:, 0], in_=wk8[:, 0])
        nc.sync.dma_start(out=xk_sb[:, :, 0:TB], in_=xk8[:, :, 0:TB])
        nc.sync.dma_start(out=wq_sb[:, 0], in_=wq8[:, 0])
        nc.sync.dma_start(out=xq_sb[:, :, 0:TB], in_=xq8[:, :, 0:TB])
        nc.sync.dma_start(out=bq_sb, in_=bq)
        for i in range(1, NTB):
            sl = slice(i * TB, (i + 1) * TB)
            nc.sync.dma_start(out=xk_sb[:, :, sl], in_=xk8[:, :, sl])
        nc.sync.dma_start(out=wv_sb, in_=wv16)
        nc.sync.dma_start(out=bv_sb, in_=bv16)
        for i in range(NTB):
            sl = slice(i * TB, (i + 1) * TB)
            nc.sync.dma_start(out=xv_sb[:, :, sl], in_=xv16[:, :, sl])
        nc.sync.dma_start(out=id_sb, in_=ident)
        nc.sync.dma_start(out=wk_sb[:, 1], in_=wk8[:, 1])
        nc.sync.dma_start(out=wq_sb[:, 1], in_=wq8[:, 1])
        for i in range(1, NTB):
            sl = slice(i * TB, (i + 1) * TB)
            nc.sync.dma_start(out=xq_sb[:, :, sl], in_=xq8[:, :, sl])
        nc.sync.dma_start(out=wo_sb, in_=wo16)
        nc.sync.dma_start(out=bo_sb, in_=bo16)
        nc.vector.memset(vaug[:, :, :, D:D + 1], 1.0)
        if F32T:
            nc.vector.tensor_copy(out=id32_sb, in_=id_sb)

        def qk_group(ps, w_sb, x_sb, dstT, m, nb, has_bias):
            g, slab = m // 2, m % 2
            half, mh = m // 2, m % 2
            tsl = slice(nb * TB, (nb + 1) * TB)
            if NODR_PROJ or QK16:
                for kt in range(KT):
                    nc.tensor.matmul(
                        ps,
                        (w_sb[:, half, kt, mh * 128:(mh + 1) * 128]),
                        (x_sb[:, kt, tsl]),
                        start=(kt == 0),
                        stop=(kt == KT - 1),
                    )
            else:
                for k2 in range(KT // 2):
                    nc.tensor.matmul(
                        ps,
                        (w_sb[:, half, 2 * k2:2 * k2 + 2,
                              mh * 128:(mh + 1) * 128]),
                        (x_sb[:, 2 * k2:2 * k2 + 2, tsl]),
                        start=(k2 == 0),
                        stop=(k2 == KT // 2 - 1),
                        perf_mode=DR,
                    )
            eng = nc.gpsimd if QKDRAIN_POOL else nc.vector
            if has_bias:
                eng.tensor_scalar(
                    out=dstT[:, g, slab, tsl], in0=ps,
                    scalar1=bq_sb[:, m:m + 1], scalar2=None, op0=ALU.add,
                )
            else:
                eng.tensor_copy(out=dstT[:, g, slab, tsl], in_=ps)

        # ---------------- Phase A: upfront projections ----------------
        # Just k(g0, nb0) + q(g0, nb0): enough for sblock 0's first t-tiles.
        with tc.tile_pool(name="pa_ps", bufs=3, space="PSUM") as ppool:
            for m in range(2):
                ps = ppool.tile([128, TB], F32, tag="pp")
                qk_group(ps, wk_sb, xk_sb, kT8, m, 0, False)
            for m in range(2):
                ps = ppool.tile([128, TB], F32, tag="pp")
                qk_group(ps, wq_sb, xq_sb, qT8, m, 0, True)

        # ---------------- Phase B ----------------
        with tc.tile_pool(name="sc3p", bufs=1, space="PSUM") as sc3p, \
             tc.tile_pool(name="sc2p", bufs=1, space="PSUM") as sc2p, \
             tc.tile_pool(name="accp", bufs=1, space="PSUM") as accp, \
             tc.tile_pool(name="flxp", bufs=1, space="PSUM") as flxp, \
             tc.tile_pool(name="exs", bufs=(EXBUFS + 1) // 2) as expool, \
             tc.tile_pool(name="ex2s", bufs=(EXBUFS + 1) // 2) as ex2pool, \
             tc.tile_pool(name="aon", bufs=2) as aopool, \
             tc.tile_pool(name="rcs", bufs=2) as rcpool, \
             tc.tile_pool(name="osg", bufs=2) as ospool:
            sc3 = sc3p.tile([128, 3, 512], F32, name="sc3")
            sc2 = sc2p.tile([128, 2, 512], F32, name="sc2")
            acc = accp.tile([128, 4, D + 1], F32, name="acc")
            acc_flat = acc.rearrange("p q x -> p (q x)")
            flex = [
                flxp.tile([128, 512], F32, name="flexA"),
                flxp.tile([128, 512], F32, name="flexB"),
            ]
            fpar = [0]  # flex parity counter
            tail_mode = [False]
            front_mode = [True]   # until first attn@v uses the acc bank

            def next_flex(big=False):
                if tail_mode[0]:
                    ring = [flex[0], flex[1], sc2[:, 0, :], sc2[:, 1, :],
                            acc_flat]
                elif front_mode[0]:
                    ring = [flex[0], flex[1], acc_flat]
                else:
                    ring = [flex[0], flex[1]]
                f = ring[fpar[0] % len(ring)]
                fpar[0] += 1
                while big and f.free_size() < 512:
                    f = ring[fpar[0] % len(ring)]
                    fpar[0] += 1
                return f

            ex_tiles = {}
            aon_tiles = {}
            ostage = {}
            deferred = deque()

            def emit_paced(kind, args):
                if kind == "vproj":
                    cb, mt = args
                    f = next_flex()
                    ps = f[:, 0:128]
                    for kt in range(KT):
                        nc.tensor.matmul(
                            ps,
                            (xv_sb[:, kt, mt * 128:(mt + 1) * 128]),
                            (wv_sb[:, kt, cb * 128:(cb + 1) * 128]),
                            start=(kt == 0),
                            stop=(kt == KT - 1),
                        )
                    nc.vector.tensor_add(
                        out=vaug[:, mt, 2 * cb:2 * cb + 2, 0:D],
                        in0=ps.rearrange("p (h d) -> p h d", d=D),
                        in1=bv_sb[:, cb * 128:(cb + 1) * 128].rearrange(
                            "p (h d) -> p h d", d=D
                        ),
                    )
                elif kind == "qproj":
                    m, nb = args
                    qk_group(next_flex(big=True), wq_sb, xq_sb, qT8, m, nb, True)
                else:  # kproj
                    m, nb = args
                    qk_group(next_flex(big=True), wk_sb, xk_sb, kT8, m, nb, False)

            def do_transp(q, cb, aon, sb):
                if F32T:
                    tp = next_flex()[:, 0:128]
                    nc.tensor.transpose(
                        tp, (aon[:, q, cb * 2:(cb + 1) * 2, :]), (id32_sb)
                    )
                else:
                    tp = next_flex()[:, 0:64].bitcast(BF16)
                    nc.tensor.transpose(
                        tp, (aon[:, q, cb * 2:(cb + 1) * 2, :]), (id_sb)
                    )
                nc.vector.tensor_copy(
                    out=aoT[:, cb, sb % 2, q * 128:(q + 1) * 128],
                    in_=tp,
                )

            def emit_attnv(ci, gts):
                ex = ex_tiles.pop(ci)
                for j, gt in enumerate(gts):
                    u, tt = gt // NTT, gt % NTT
                    sb, h = u // NHC, u % NHC
                    for q in range(4):
                        nc.tensor.matmul(
                            acc[:, q, :],
                            (ex[:, j, q * 128:(q + 1) * 128]),
                            (vaug[:, tt, h, :]),
                            start=(tt == 0 and q == 0),
                            stop=(tt == NTT - 1 and q == 3),
                            skip_group_check=True,
                        )
                    if tt == NTT - 1:
                        finish_unit(u)

            def finish_unit(u):
                sb, h = u // NHC, u % NHC
                rc = rcpool.tile([128, 4, 1], F32, tag="rc", name="rc")
                nc.vector.reciprocal(out=rc, in_=acc[:, :, D:D + 1])
                aon = aon_tiles[sb % 2]
                for q in range(4):
                    nc.vector.tensor_scalar(
                        out=aon[:, q, h, :],
                        in0=acc[:, q, 0:D],
                        scalar1=rc[:, q, :],
                        scalar2=None,
                        op0=ALU.mult,
                    )
                if h % 2 == 1 and h != NHC - 1:
                    cb = h // 2
                    for q in range(4):
                        def t_transp(q=q, cb=cb, aon=aon, sb=sb):
                            do_transp(q, cb, aon, sb)
                        deferred.append(t_transp)
                if h == NHC - 1:
                    schedule_outproj(sb, aon)

            def schedule_outproj(sb, aon):
                for mt in range(4):
                    t0 = sb * 512 + mt * 128

                    def t_transp3(q=mt, aon=aon, sb=sb):
                        do_transp(q, NHC // 2 - 1, aon, sb)
                    deferred.append(t_transp3)
                    for ep in range(E // 256):
                        def t_oproj(mt=mt, ep=ep, t0=t0, sb=sb):
                            if ep == 0:
                                ostage[mt % 2] = ospool.tile(
                                    [128, E], F32, tag="os", name="os"
                                )
                            op = next_flex()[:, 0:256]
                            for half in range(2):
                                ec = ep * 2 + half
                                for kc in range(NM):
                                    nc.tensor.matmul(
                                        op[:, half * 128:(half + 1) * 128],
                                        (aoT[:, kc, sb % 2,
                                             mt * 128:(mt + 1) * 128]),
                                        (wo_sb[:, kc, ec * 128:(ec + 1) * 128]),
                                        start=(kc == 0 and half == 0),
                                        stop=(kc == NM - 1 and half == 1),
                                        skip_group_check=True,
                                    )
                            nc.vector.tensor_add(
                                out=ostage[mt % 2][:, ep * 256:(ep + 1) * 256],
                                in0=op,
                                in1=bo_sb[:, ep * 256:(ep + 1) * 256],
                            )
                            if ep % 2 == 1:
                                eh = ep // 2
                                os_t = ostage[mt % 2]

                                def t_dma(os_t=os_t, eh=eh, t0=t0):
                                    nc.sync.dma_start(
                                        out=out[t0:t0 + 128,
                                                eh * 512:(eh + 1) * 512],
                                        in_=os_t[:, eh * 512:(eh + 1) * 512],
                                    )
                                deferred.append(t_dma)
                        deferred.append(t_oproj)

            pending = deque()
            v_emitted = set()

            def attnv_ready(ci):
                return all(
                    ((gt // NTT % NHC) // 2, gt % NTT) in v_emitted
                    for gt in chunks[ci]
                )

            for ci, gts in enumerate(chunks):
                sc = sc3 if len(gts) == 3 else sc2
                for j, gt in enumerate(gts):
                    u, tt = gt // NTT, gt % NTT
                    sb, h = u // NHC, u % NHC
                    if h == 0 and tt == 0:
                        aon_tiles[sb % 2] = aopool.tile(
                            [128, 4, NHC, D], F32 if F32T else BF16,
                            tag="aon", name="aon"
                        )
                    g, hb = h // 4, h % 4
                    p0 = hb * 32
                    if NODR_SCORES or QK16:
                        for slab in range(2):
                            nc.tensor.matmul(
                                sc[:, j, :],
                                (kT8[p0:p0 + 32, g, slab,
                                     tt * 128:(tt + 1) * 128]),
                                (qT8[p0:p0 + 32, g, slab,
                                     sb * 512:(sb + 1) * 512]),
                                start=(slab == 0),
                                stop=(slab == 1),
                                tile_position=(p0, 0),
                            )
                    else:
                        nc.tensor.matmul(
                            sc[:, j, :],
                            (kT8[p0:p0 + 32, g, :, tt * 128:(tt + 1) * 128]),
                            (qT8[p0:p0 + 32, g, :, sb * 512:(sb + 1) * 512]),
                            start=True,
                            stop=True,
                            perf_mode=DR,
                            tile_position=(p0, 0),
                        )
                n = len(gts)
                if n == 3:
                    ex = expool.tile([128, 3, 512], BF16, tag="ex", name="ex")
                else:
                    ex = ex2pool.tile([128, 2, 512], BF16, tag="ex2", name="ex2")
                nc.scalar.activation(
                    out=ex[:, 0:n, :], in_=sc[:, 0:n, :], func=AF.Exp, scale=0.125
                )
                ex_tiles[ci] = ex
                # attn@v for pending chunks whose v-deps are emitted
                # (cap catch-up at 2 chunks per step to avoid PE bursts)
                if ci > 0:
                    pending.append(ci - 1)
                drained = 0
                while pending and attnv_ready(pending[0]) and (
                    drained < 2 or ci - pending[0] >= EXBUFS - 3
                ):
                    front_mode[0] = False
                    emit_attnv(pending[0], chunks[pending[0]])
                    pending.popleft()
                    drained += 1
                # paced projection work assigned to this chunk
                for kind, args in emit_at[ci]:
                    emit_paced(kind, args)
                    if kind == "vproj":
                        v_emitted.add(args)
                if pending:
                    assert ci - pending[0] < EXBUFS - 1, (
                        f"ex ring overrun at chunk {ci}: pending {pending[0]}"
                    )
                k = max(0, DRAIN - len(emit_at[ci]))
                for _ in range(min(k, len(deferred))):
                    deferred.popleft()()
            if DBG:
                nc.sync.dma_start(out=d_qT, in_=qT8)
                nc.sync.dma_start(out=d_kT, in_=kT8)
                nc.sync.dma_start(out=d_vaug, in_=vaug)
            tail_mode[0] = True
            pending.append(NCH - 1)
            while pending:
                emit_attnv(pending[0], chunks[pending[0]])
                pending.popleft()
            while deferred:
                deferred.popleft()()
            if DBG:
                nc.sync.dma_start(out=d_aoT, in_=aoT)

    nc.compile()
    return nc


_PROG = {}


def _get_prog(num_devices=8):
    if num_devices not in _PROG:
        _PROG[num_devices] = build_program(num_devices)
    return _PROG[num_devices]


FP8NP = ml_dtypes.float8_e4m3
BF16NP = ml_dtypes.bfloat16
QKNP = BF16NP if QK16 else FP8NP


def _tile_x(x2d, dt):
    # [S, E] -> [128, KT, S]; element (p, k, t) = x2d[t, k*128+p]
    xt = np.ascontiguousarray(x2d.T)
    return np.ascontiguousarray(
        xt.reshape(KT, 128, S).transpose(1, 0, 2).astype(dt)
    )


def _tile_w(weff_cols, dt):
    # weff_cols [E, CH] -> [128, KT, CH]; (p, k, c) = weff_cols[k*128+p, c]
    return np.ascontiguousarray(
        weff_cols.reshape(KT, 128, CH).transpose(1, 0, 2).astype(dt)
    )


def _tile_w_half(weff_cols, dt):
    # [E, CH] -> [128, 2, KT, CH//2]; (p, half, k, j) = W[k*128+p, half*256+j]
    wt = weff_cols.reshape(KT, 128, CH).transpose(1, 0, 2)  # [128, KT, CH]
    return np.ascontiguousarray(
        wt.reshape(128, KT, 2, CH // 2).transpose(0, 2, 1, 3).astype(dt)
    )


def _qk_perm():
    # channel permutation: m-tile m=(g,slab), partition p=(hb*32+r) holds
    # local channel (g*4+hb)*64 + slab*32 + r
    j = np.arange(CH)
    m, p = j // 128, j % 128
    g, slab = m // 2, m % 2
    hb, r = p // 32, p % 32
    return (g * 4 + hb) * 64 + slab * 32 + r


def prep_in_maps(x_q, x_k, x_v, Wq, bq, Aq, Bq, Wk, bk, Wv, bv, Av, Bv, Wo, bo):
    x_q = np.asarray(x_q, np.float32)
    x_k = np.asarray(x_k, np.float32)
    x_v = np.asarray(x_v, np.float32)
    scaling = 2.0  # lora_alpha / r
    wq_eff = (np.asarray(Wq).T + (np.asarray(Aq) @ np.asarray(Bq)) * scaling).astype(
        np.float32
    )
    wv_eff = (np.asarray(Wv).T + (np.asarray(Av) @ np.asarray(Bv)) * scaling).astype(
        np.float32
    )
    wk_T = np.asarray(Wk).T.astype(np.float32)  # bk dropped (softmax-invariant)
    bq = np.asarray(bq, np.float32)
    bv = np.asarray(bv, np.float32)
    bo = np.asarray(bo, np.float32)
    woT = np.ascontiguousarray(np.asarray(Wo).T.astype(np.float32))
    perm = _qk_perm()
    ident = np.eye(128, dtype=BF16NP)

    nbatch = x_q.shape[1]
    in_maps = []
    for c in range(2 * nbatch):
        b = c // 2
        hg = c % 2
        ch0 = hg * CH
        in_maps.append({
            "xq8": _tile_x(x_q[:, b, :], QKNP),
            "xk8": _tile_x(x_k[:, b, :], QKNP),
            "xv16": _tile_x(x_v[:, b, :], BF16NP),
            "wq8": _tile_w_half(wq_eff[:, ch0 + perm], QKNP),
            "wk8": _tile_w_half(wk_T[:, ch0 + perm], QKNP),
            "wv16": _tile_w(wv_eff[:, ch0:ch0 + CH], BF16NP),
            "wo16": np.ascontiguousarray(
                woT[ch0:ch0 + CH, :].reshape(NM, 128, E).transpose(1, 0, 2)
            ).astype(BF16NP),
            "bq": np.ascontiguousarray(bq[ch0 + perm].reshape(NM, 128).T),
            "bv16": np.ascontiguousarray(
                np.broadcast_to(bv[ch0:ch0 + CH], (128, CH))
            ).astype(BF16NP),
            "bo16": (
                np.ascontiguousarray(np.broadcast_to(bo, (128, E))).astype(BF16NP)
                if hg == 0
                else np.zeros((128, E), BF16NP)
            ),
            "ident": ident,
        })
    return in_maps


def gather_out(results, nbatch):
    return np.stack(
        [
            np.asarray(results[2 * b]["out"], np.float32)
            + np.asarray(results[2 * b + 1]["out"], np.float32)
            for b in range(nbatch)
        ],
        axis=1,
    )


def kernel(**inputs):
    nc = _get_prog(NCORES)
    in_maps = prep_in_maps(**inputs)
    res = run_bass_kernel_spmd(nc, in_maps, core_ids=list(range(NCORES)))
    return gather_out(res.results, B)
